# revision 37
# baseline (speedup 1.0000x reference)
"""BrainGCN kernel for 8 Trainium2 NeuronCores (Bass/Tile).

Strategy (v3):
- Nodes are degree-sorted and snake-dealt across 8 cores; within a core,
  positions are (degree, stage0-count)-ascending into 49 chunks of 128.
  Chunks are grouped into superchunks of 4 (512 PSUM columns).
- conv1: the host pre-expands x*dinv into per-edge-slot columns (feature
  major, bf16). The device streams slabs and accumulates W1^T @ x_slot
  directly in PSUM across rounds (no DVE reduction at all). tanh/bias/dinv
  applied per superchunk, z2' = (h1*dinv) @ W2 rows written out immediately.
- The bf16 z2' table is AllGathered in two stages (low-degree chunks first,
  so stage 0 ships mid-conv1 and the stage-1 AllGather is hidden under
  stage-0 gathers). Table rows are PAIRS (nodes 2r, 2r+1 -> 256B), so
  dma_gather reads the bf16 table directly (no f32 expansion) and int16
  pair indices reach both stage tables.
- conv2: dma_gather per-edge pair rows, one in-place copy_predicated picks
  the wanted half via a parity mask, then wide strided DVE adds into the
  round-structured accumulator. h2 + the FC head run per 4-chunk group,
  emitted right after the gather group that completes those chunks, so the
  tail is just the last group's chain.

kernel(**inputs) takes FULL inputs, preprocesses + shards on host, compiles
and runs the SPMD program on cores 0..7, and reassembles the full output.
"""

import os
import warnings

warnings.filterwarnings("ignore")

import numpy as np
import ml_dtypes

from concourse import bacc, bass, mybir, tile
from concourse.masks import make_identity
import concourse.bass_utils as bass_utils

P = 128
NCORES = 8
NCH = 49          # chunks per core
# staged table chunk bounds (each stage = contiguous chunk range, superchunk
# aligned, 8*chunks*128 < 32768 rows so int16 gather indices reach them)
SBOUNDS = (0, 20, 49)
NSTG = len(SBOUNDS) - 1
GQ = int(os.environ.get("GCN_GQ", "2"))
GBLK = int(os.environ.get("GCN_GBLK", "8"))   # blocks per dma_gather
SLAB = 4096       # x_exp columns per DMA slab

X_DT = os.environ.get("GCN_XDT", "bf16")      # x_exp dtype: bf16 | fp8
TAB_DT = os.environ.get("GCN_TABDT", "bf16")  # z2 pair-table dtype (bf16: 256B pair rows)

_NPDT = {
    "f32": np.float32,
    "bf16": ml_dtypes.bfloat16,
    "fp8": ml_dtypes.float8_e4m3,
}
_MYDT = {
    "f32": mybir.dt.float32,
    "bf16": mybir.dt.bfloat16,
    "fp8": mybir.dt.float8e4,
}


# ---------------------------------------------------------------------------
# Host preprocessing
# ---------------------------------------------------------------------------

def _preprocess(x, edge_index):
    N = x.shape[0]
    E = edge_index.shape[1]
    src = np.asarray(edge_index[0], dtype=np.int64)
    dst = np.asarray(edge_index[1], dtype=np.int64)

    shard = NCH * P                      # 6272
    percore = N // NCORES                # 6250 (N divisible by 8)
    assert percore * NCORES == N
    assert percore <= shard - NSTG, "need a pad row per stage"
    Hs = [(SBOUNDS[s + 1] - SBOUNDS[s]) * P for s in range(NSTG)]
    assert all(NCORES * h < 32768 for h in Hs)

    deg = 1 + np.bincount(dst, minlength=N)          # includes self-loop
    dinv = (1.0 / np.sqrt(deg)).astype(np.float32)

    # --- snake deal nodes (degree desc) to cores -> equal counts, matched
    # degree profiles
    order = np.argsort(-deg, kind="stable")
    i = np.arange(N)
    pos8 = i % NCORES
    pair = (i // NCORES) % 2
    core_seq = np.where(pair == 0, pos8, NCORES - 1 - pos8)
    core_of = np.empty(N, np.int32)
    core_of[order] = core_seq.astype(np.int32)

    # --- within-core positions: degree ascending; chunk permutation so that
    # within each superchunk of 4 chunks the per-chunk max degree DESCENDS
    # (live chunks at round k form a prefix of the psum tile).
    # prelim chunk q (deg asc) -> device chunk l: reverse within group of 4.
    q_ids = np.arange(NCH)
    g = q_ids // 4
    glen = np.minimum(4, NCH - 4 * g)
    l_of_q = 4 * g + (glen - 1) - (q_ids - 4 * g)

    # slot list in (q asc, lane asc) order, skipping one hole position per
    # stage (device (l=stage_end-1, lane=127) stays empty -> zero table row
    # for padding).
    hole_q = [int(np.where(l_of_q == SBOUNDS[s + 1] - 1)[0][0]) for s in range(NSTG)]
    slot_q = np.repeat(q_ids, P)
    slot_lane = np.tile(np.arange(P), NCH)
    is_hole = np.zeros(len(slot_q), bool)
    for hq in hole_q:
        is_hole |= (slot_q == hq) & (slot_lane == P - 1)
    keep = ~is_hole
    slot_q = slot_q[keep]
    slot_lane = slot_lane[keep]
    slot_l = l_of_q[slot_q]

    l_of = np.empty(N, np.int32)
    lane_of = np.empty(N, np.int32)
    node_at = np.full((NCORES, shard), -1, np.int64)

    def assign_positions(secondary):
        """Place nodes per core ordered by (deg asc, secondary asc)."""
        for c in range(NCORES):
            sel = order[core_of[order] == c]
            key = np.lexsort((secondary[sel], deg[sel]))
            sel = sel[key]
            ls = slot_l[: len(sel)]
            lanes = slot_lane[: len(sel)]
            l_of[sel] = ls
            lane_of[sel] = lanes
            node_at[c, ls * P + lanes] = sel
        return (np.searchsorted(SBOUNDS, l_of, side="right") - 1).astype(np.int8)

    # iteration 1: by degree only -> provisional stages
    stage_of = assign_positions(np.zeros(N, np.int64))
    # iterations 2-3: secondary-sort by stage-0 source count so chunks are
    # homogeneous in per-stage counts (tight conv2 round padding)
    for _ in range(2):
        c0 = np.bincount(dst[stage_of[src] == 0], minlength=N) + (stage_of == 0)
        stage_of = assign_positions(c0)
    pos_of = l_of.astype(np.int64) * P + lane_of

    # --- conv1 block structure: superchunk round-major, live prefix
    # per-chunk max total degree (self included) across all cores
    K1 = np.zeros(NCH, np.int32)
    for c in range(NCORES):
        sel = node_at[c][node_at[c] >= 0]
        np.maximum.at(K1, l_of[sel], deg[sel].astype(np.int32))

    n_sc = (NCH + 3) // 4
    sc_chunks = [list(range(4 * s, min(NCH, 4 * s + 4))) for s in range(n_sc)]
    # per-superchunk rounds: (col0, live) ; colstart2d[k, l] -> x_exp col base
    colstart2d = np.full((int(K1.max()), NCH), -1, np.int64)
    sc_rounds = []   # per sc: list of (col0, ncols)
    cursor = 0
    for s, chs in enumerate(sc_chunks):
        kmax = int(K1[chs[0]])
        assert all(K1[chs[j]] <= K1[chs[j - 1]] for j in range(1, len(chs)))
        rounds = []
        for k in range(kmax):
            live = sum(1 for ch in chs if K1[ch] > k)
            for j in range(live):
                colstart2d[k, chs[j]] = cursor + j * P
            rounds.append((cursor, live * P))
            cursor += live * P
        sc_rounds.append(rounds)
    S1 = cursor

    # slabs: pack rounds into DMA ranges <= SLAB cols; map rounds -> slab;
    # force a slab break at stage boundaries
    stage_end_sc = {SBOUNDS[s + 1] // 4 - 1 for s in range(NSTG)}
    slabs = []      # (col0, col1)
    round_slab = []  # per sc: list of slab idx for each round
    cur0, cur1 = 0, 0
    for s, rounds in enumerate(sc_rounds):
        rs = []
        for (c0, nc_) in rounds:
            if c0 + nc_ - cur0 > SLAB:
                if cur1 > cur0:
                    slabs.append((cur0, cur1))
                cur0 = c0
            cur1 = c0 + nc_
            rs.append(len(slabs))
        round_slab.append(rs)
        if s in stage_end_sc and cur1 > cur0:
            slabs.append((cur0, cur1))
            cur0 = cur1
    if cur1 > cur0:
        slabs.append((cur0, cur1))

    # --- conv2 structure: per-stage round-major blocks
    # counts of stage-s sources per node (self counts in own stage)
    cnt = np.zeros((NSTG, N), np.int64)
    np.add.at(cnt, (stage_of[src], dst), 1)
    for s in range(NSTG):
        cnt[s] += (stage_of == s)
    K2 = np.zeros((NSTG, NCH), np.int32)
    for c in range(NCORES):
        sel = node_at[c][node_at[c] >= 0]
        for s in range(NSTG):
            np.maximum.at(K2[s], l_of[sel], cnt[s][sel].astype(np.int32))

    blocks2 = {s: [] for s in range(NSTG)}
    for s in range(NSTG):
        for k in range(int(K2[s].max())):
            for l in range(NCH):
                if K2[s, l] > k:
                    blocks2[s].append((k, l))
    b2_of = {s: {} for s in range(NSTG)}
    barr = {}
    for s in range(NSTG):
        barr[s] = np.full((max(1, int(K2[s].max())), NCH), -1, np.int64)
        for i_, (k, l) in enumerate(blocks2[s]):
            b2_of[s][(k, l)] = i_
            barr[s][k, l] = i_
    groups = []   # (stage, [block list])
    for s in range(NSTG):
        for i_ in range(0, len(blocks2[s]), GBLK):
            groups.append((s, blocks2[s][i_: i_ + GBLK]))

    slots2 = sum(len(blocks2[s]) for s in range(NSTG)) * P
    per_core_work = (E + N) / NCORES
    print(
        f"[pre] conv1 slots={S1} ({S1/per_core_work:.3f}x) "
        f"conv2 slots={slots2} ({slots2/per_core_work:.3f}x) "
        f"groups={len(groups)} slabs={len(slabs)}"
    )

    # global table rows
    Hs_arr = np.array(Hs)
    base_l = np.array(SBOUNDS[:-1])
    srow = core_of.astype(np.int64) * Hs_arr[stage_of] + (
        (l_of - base_l[stage_of]).astype(np.int64) * P + lane_of
    )

    # --- per-core arrays (vectorized fills)
    xs = x.astype(np.float32) * dinv[:, None]
    xdt = _NPDT[X_DT]
    xsT = np.ascontiguousarray(xs.T).astype(xdt)      # [128, N]

    # edge ranks within destination (self-loop is k=0)
    eorder = np.argsort(dst, kind="stable")
    dst_s = dst[eorder]
    src_s = src[eorder]
    estarts = np.searchsorted(dst_s, np.arange(N))
    k1_e = 1 + np.arange(E) - estarts[dst_s]          # conv1 round of each edge

    # conv2 ranks: within (dst, stage(src)); self first in its own stage
    sstage = stage_of[src_s]
    k2_e = np.empty(E, np.int64)
    for s in range(NSTG):
        m = sstage == s
        d = dst_s[m]
        # rank within equal-d runs (d is sorted)
        startmask = np.ones(len(d), bool)
        startmask[1:] = d[1:] != d[:-1]
        runstart = np.where(startmask, np.arange(len(d)), 0)
        np.maximum.accumulate(runstart, out=runstart)
        k2_e[m] = np.arange(len(d)) - runstart
    # shift by one where the self node is in the same stage as the source
    k2_e += (stage_of[dst_s] == sstage)

    per_core = []
    for c in range(NCORES):
        m = core_of[dst_s] == c
        de, se = dst_s[m], src_s[m]

        # conv1 x_exp
        x_exp = np.zeros((P, S1), dtype=xdt)
        cols_e = colstart2d[k1_e[m], l_of[de]] + lane_of[de]
        own = node_at[c][node_at[c] >= 0]
        cols_self = colstart2d[0, l_of[own]] + lane_of[own]
        x_exp[:, cols_self] = xsT[:, own]
        x_exp[:, cols_e] = xsT[:, se]

        # conv2 idx arrays: gather PAIR rows (256B of bf16 = nodes 2r,2r+1);
        # parity mask selects the wanted half on-device
        idx_parts = []
        par_parts = []
        for s in range(NSTG):
            nb = len(blocks2[s])
            zero_row = c * Hs[s] + Hs[s] - 1
            idxflat = np.full(nb * P, zero_row, np.int64)
            ms = stage_of[se] == s
            bidx = barr[s][k2_e[m][ms], l_of[de[ms]]]
            assert (bidx >= 0).all()
            idxflat[bidx * P + lane_of[de[ms]]] = srow[se[ms]]
            owns = own[stage_of[own] == s]
            bown = barr[s][0, l_of[owns]]
            assert (bown >= 0).all()
            idxflat[bown * P + lane_of[owns]] = srow[owns]
            idx_parts.append(idxflat // 2)
            par_parts.append((idxflat % 2).astype(np.int8))
            assert idx_parts[-1].max() < 32768

        # wrap idx per group: [16, S] slabs concatenated; masks [128, blocks]
        slabs_i = []
        masks_i = []
        for s, blks in groups:
            i0 = b2_of[s][blks[0]]
            flat = idx_parts[s][i0 * P: (i0 + len(blks)) * P]
            S = len(flat) // 16
            slabs_i.append(flat.reshape(S, 16).T.astype(np.int16))
            masks_i.append(par_parts[s][i0 * P: (i0 + len(blks)) * P].reshape(len(blks), P).T)
        idx_cat = np.concatenate(slabs_i, axis=1)
        idx_rep = np.tile(idx_cat, (8, 1))            # [128, sum S]
        mask_cat = np.ascontiguousarray(np.concatenate(masks_i, axis=1))  # [128, nblocks]

        dinv_loc = np.zeros(shard, np.float32)
        vmask = node_at[c] >= 0
        dinv_loc[vmask] = dinv[node_at[c][vmask]]
        dinv_fm = np.tile(dinv_loc[None, :], (64, 1)).astype(ml_dtypes.bfloat16)
        dinv_nm = dinv_loc.reshape(NCH, P).T.astype(np.float32).copy()

        per_core.append(
            dict(x_exp=x_exp, idx=idx_rep, mask=mask_cat, dinv_fm=dinv_fm,
                 dinv_nm=dinv_nm)
        )

    # FC groups of 4 consecutive chunks; emit each group's h2+FC chain right
    # after the gather group that completes its chunks
    last_group_of_chunk = np.zeros(NCH, np.int64)
    for gi, (s, blks) in enumerate(groups):
        for (k, l) in blks:
            last_group_of_chunk[l] = max(last_group_of_chunk[l], gi)
    n_fc = (NCH + 3) // 4
    fc_groups = [list(range(4 * g, min(NCH, 4 * g + 4))) for g in range(n_fc)]
    fc_done = [int(max(last_group_of_chunk[l] for l in fg)) for fg in fc_groups]
    fc_after = {}
    for g, gi in enumerate(fc_done):
        fc_after.setdefault(gi, []).append(g)

    st = dict(
        N=N, shard=shard, Hs=Hs, S1=S1,
        fc_groups=fc_groups, fc_after=fc_after,
        sc_chunks=sc_chunks, sc_rounds=sc_rounds, slabs=slabs,
        round_slab=round_slab, blocks2=blocks2, groups=groups, b2_of=b2_of,
        node_at=node_at, idx_cols=per_core[0]["idx"].shape[1],
        nblocks=per_core[0]["mask"].shape[1],
    )
    return st, per_core, dinv


# ---------------------------------------------------------------------------
# Program builder
# ---------------------------------------------------------------------------

def _segments(blks):
    """Runs of (same k, consecutive l) for wide DVE adds."""
    segs = []
    s = 0
    for i in range(1, len(blks) + 1):
        if (
            i == len(blks)
            or blks[i][0] != blks[s][0]
            or blks[i][1] != blks[i - 1][1] + 1
        ):
            segs.append((s, i))
            s = i
    return segs


def _build(st, weights, n_passes=1):
    shard = st["shard"]
    S1 = st["S1"]
    Hs = st["Hs"]
    groups = st["groups"]
    xdt = _MYDT[X_DT]
    tdt = _MYDT[TAB_DT]
    tnp = _NPDT[TAB_DT]

    fb2 = float(np.asarray(weights["fc_b2"]).reshape(-1)[0])

    nc = bacc.Bacc(
        "TRN2",
        target_bir_lowering=False,
        debug=False,
        enable_asserts=False,
        num_devices=NCORES,
        num_swdge_queues=GQ,
        dynamic_dma_scratch_size=int(os.environ.get("GCN_SCRATCH", "16384")),
    )

    x_exp_in = nc.dram_tensor("x_exp", [P, S1], xdt, kind="ExternalInput")
    idx_in = nc.dram_tensor("idx2", [P, st["idx_cols"]], mybir.dt.int16, kind="ExternalInput")
    mask_in = nc.dram_tensor("mask2", [P, st["nblocks"]], mybir.dt.int8, kind="ExternalInput")
    dinv_fm_in = nc.dram_tensor("dinv_fm", [64, shard], mybir.dt.bfloat16, kind="ExternalInput")
    dinv_nm_in = nc.dram_tensor("dinv_nm", [P, NCH], mybir.dt.float32, kind="ExternalInput")
    w1_in = nc.dram_tensor("w1", [P, 64], xdt, kind="ExternalInput")
    w2_in = nc.dram_tensor("w2", [64, 64], mybir.dt.bfloat16, kind="ExternalInput")
    fw1_in = nc.dram_tensor("fw1", [64, 32], mybir.dt.bfloat16, kind="ExternalInput")
    fw2_in = nc.dram_tensor("fw2", [32, 1], mybir.dt.bfloat16, kind="ExternalInput")
    b1_in = nc.dram_tensor("b1c", [64, 1], mybir.dt.float32, kind="ExternalInput")
    b2e_in = nc.dram_tensor("b2e", [P, 64], mybir.dt.float32, kind="ExternalInput")
    fb1_in = nc.dram_tensor("fb1c", [32, 1], mybir.dt.float32, kind="ExternalInput")
    y_out = nc.dram_tensor("y", [1, shard], mybir.dt.float32, kind="ExternalOutput")

    with tile.TileContext(nc) as tc:
        with (
            tc.tile_pool(name="const", bufs=1) as constp,
            tc.tile_pool(name="big", bufs=1) as bigp,
            tc.tile_pool(name="xslab", bufs=4) as xslabp,
            tc.tile_pool(name="gstage", bufs=5) as gstagep,
            tc.tile_pool(name="psum", bufs=1, space="PSUM") as psump,
            tc.tile_pool(name="small", bufs=2) as smallp,
            tc.tile_pool(name="dram", bufs=1, space="DRAM") as dramp,
        ):
            w1_sb = constp.tile([P, 64], xdt, name="w1_sb")
            nc.sync.dma_start(out=w1_sb[:], in_=w1_in.ap())
            w2_sb = constp.tile([64, 64], mybir.dt.bfloat16, name="w2_sb")
            nc.sync.dma_start(out=w2_sb[:], in_=w2_in.ap())
            fw1_sb = constp.tile([64, 32], mybir.dt.bfloat16, name="fw1_sb")
            nc.sync.dma_start(out=fw1_sb[:], in_=fw1_in.ap())
            fw2_sb = constp.tile([32, 1], mybir.dt.bfloat16, name="fw2_sb")
            nc.sync.dma_start(out=fw2_sb[:], in_=fw2_in.ap())
            b1_sb = constp.tile([64, 1], mybir.dt.float32, name="b1_sb")
            nc.sync.dma_start(out=b1_sb[:], in_=b1_in.ap())
            b2e_sb = constp.tile([P, 64], mybir.dt.float32, name="b2e_sb")
            nc.sync.dma_start(out=b2e_sb[:], in_=b2e_in.ap())
            fb1_sb = constp.tile([32, 1], mybir.dt.float32, name="fb1_sb")
            nc.sync.dma_start(out=fb1_sb[:], in_=fb1_in.ap())
            dinv_fm = constp.tile([64, shard], mybir.dt.bfloat16, name="dinv_fm_sb")
            nc.sync.dma_start(out=dinv_fm[:], in_=dinv_fm_in.ap())
            dinv_nm = constp.tile([P, NCH], mybir.dt.float32, name="dinv_nm_sb")
            nc.sync.dma_start(out=dinv_nm[:], in_=dinv_nm_in.ap())
            ident = constp.tile([P, P], mybir.dt.bfloat16, name="ident")
            make_identity(nc, ident[:])
            mask_sb = constp.tile([P, st["nblocks"]], mybir.dt.int8, name="mask_sb")
            nc.sync.dma_start(out=mask_sb[:], in_=mask_in.ap())


            for pas in range(n_passes):
                # ============ conv1: stream + PSUM accumulate ============
                h1s = bigp.tile([64, shard], mybir.dt.bfloat16, name=f"h1s_{pas}", tag="h1s")
                # pair rows: row r holds nodes (2r, 2r+1), 128 bf16 = 256B
                ag_in = {
                    s: dramp.tile([Hs[s] // 2, 128], tdt, name=f"ag{s}i_{pas}", tag=f"ag{s}i")
                    for s in range(NSTG)
                }
                tabg = {
                    s: dramp.tile([NCORES * Hs[s] // 2, 128], tdt, name=f"t{s}g_{pas}",
                                  tag=f"t{s}g", addr_space="Shared")
                    for s in range(NSTG)
                }

                slabs = st["slabs"]
                slab_tiles = {}

                def get_slab(si):
                    if si not in slab_tiles:
                        c0, c1 = slabs[si]
                        t = xslabp.tile([P, SLAB], xdt, tag="xsl", name=f"xsl_{pas}_{si}")
                        nc.sync.dma_start(out=t[:, : c1 - c0], in_=x_exp_in.ap()[:, c0:c1])
                        slab_tiles[si] = t
                    return slab_tiles[si]

                def emit_z2(s):
                    chs = st["sc_chunks"][s]
                    w = len(chs) * P
                    pz = psump.tile([P, 256], mybir.dt.float32, tag="pz", bufs=1,
                                    name=f"pz_{pas}_{s}")
                    for j, ch in enumerate(chs):
                        nc.tensor.matmul(
                            pz[:, j * 64: (j + 1) * 64],
                            lhsT=h1s[:, ch * P: (ch + 1) * P],
                            rhs=w2_sb[:],
                            start=True, stop=True,
                        )
                    z2sb = smallp.tile([P, 256], tdt, tag="z2sb", name=f"z2sb_{pas}_{s}")
                    nc.scalar.copy(out=z2sb[:, : len(chs) * 64], in_=pz[:, : len(chs) * 64])
                    stg_id = next(i for i in range(NSTG) if chs[0] < SBOUNDS[i + 1])
                    row0 = (chs[0] - SBOUNDS[stg_id]) * P
                    nc.sync.dma_start(
                        out=ag_in[stg_id][row0 // 2: (row0 + w) // 2, :].rearrange(
                            "(c ph) (two f) -> (ph two) c f", ph=64, two=2
                        ),
                        in_=z2sb[:, : len(chs) * 64].rearrange("p (c f) -> p c f", f=64),
                    )

                pending = None
                for s, chs in enumerate(st["sc_chunks"]):
                    w = len(chs) * P
                    pt = psump.tile([64, 512], mybir.dt.float32, tag="acc", bufs=2,
                                    name=f"acc_{pas}_{s}")
                    rounds = st["sc_rounds"][s]
                    nr = len(rounds)
                    for k, (c0, ncol) in enumerate(rounds):
                        si = st["round_slab"][s][k]
                        xsl = get_slab(si)
                        s0 = slabs[si][0]
                        nc.tensor.matmul(
                            pt[:, :ncol],
                            lhsT=w1_sb[:],
                            rhs=xsl[:, c0 - s0: c0 - s0 + ncol],
                            start=(k == 0),
                            stop=(k == nr - 1),
                        )
                    if pending is not None:
                        emit_z2(pending)
                        pending = None
                    base = chs[0] * P
                    tmp = smallp.tile([64, 512], mybir.dt.float32, tag="tmp1",
                                      name=f"tmp1_{pas}_{s}")
                    nc.vector.tensor_mul(tmp[:, :w], pt[:, :w], dinv_fm[:, base: base + w])
                    nc.scalar.activation(
                        tmp[:, :w], tmp[:, :w], mybir.ActivationFunctionType.Tanh,
                        bias=b1_sb[:, :1],
                    )
                    nc.vector.tensor_mul(
                        h1s[:, base: base + w], tmp[:, :w], dinv_fm[:, base: base + w]
                    )
                    if chs[-1] + 1 in SBOUNDS:
                        emit_z2(s)
                        sid = SBOUNDS.index(chs[-1] + 1) - 1
                        nc.gpsimd.collective_compute(
                            "AllGather",
                            mybir.AluOpType.bypass,
                            replica_groups=[list(range(NCORES))],
                            ins=[ag_in[sid].opt()],
                            outs=[tabg[sid].opt()],
                        )
                    else:
                        pending = s

                # scheduler fence: keep conv2 work out of the conv1 streams
                tc.no_sync_barrier()

                # ============ conv2: gather + DVE reduce ============
                acc2 = bigp.tile([P, NCH * 64], mybir.dt.float32, name=f"acc2_{pas}", tag="acc2")
                nc.gpsimd.memset(acc2[:], 0.0)

                h2 = bigp.tile([P, NCH * 64], mybir.dt.bfloat16, name=f"h2_{pas}", tag="h2")
                h2fm = bigp.tile([64, shard], mybir.dt.bfloat16, name=f"h2fm_{pas}", tag="h2fm")
                h3 = bigp.tile([32, shard], mybir.dt.bfloat16, name=f"h3_{pas}", tag="h3")

                def emit_fc(g):
                    chs_f = st["fc_groups"][g]
                    nch_f = len(chs_f)
                    l0 = chs_f[0]
                    a0 = l0 * 64
                    w64 = nch_f * 64
                    m0 = l0 * P
                    wp = nch_f * P
                    # h2 = tanh(acc2*dinv_nm + b2) on this chunk range
                    nc.vector.tensor_mul(
                        acc2[:, a0: a0 + w64].rearrange("p (c f) -> p c f", f=64),
                        acc2[:, a0: a0 + w64].rearrange("p (c f) -> p c f", f=64),
                        dinv_nm[:, l0: l0 + nch_f, None].to_broadcast([P, nch_f, 64]),
                    )
                    nc.vector.tensor_add(
                        acc2[:, a0: a0 + w64].rearrange("p (c f) -> p c f", f=64),
                        acc2[:, a0: a0 + w64].rearrange("p (c f) -> p c f", f=64),
                        b2e_sb[:, None, :].to_broadcast([P, nch_f, 64]),
                    )
                    nc.scalar.activation(
                        h2[:, a0: a0 + w64], acc2[:, a0: a0 + w64],
                        mybir.ActivationFunctionType.Tanh,
                    )
                    for ch in chs_f:
                        ptr = psump.tile([64, P], mybir.dt.bfloat16, tag="pst", bufs=2,
                                         name=f"pst_{pas}_{ch}")
                        nc.tensor.transpose(
                            out=ptr[:], in_=h2[:, ch * 64: (ch + 1) * 64],
                            identity=ident[:],
                        )
                        nc.scalar.copy(out=h2fm[:, ch * P: (ch + 1) * P], in_=ptr[:])
                    pf = psump.tile([32, 512], mybir.dt.float32, tag="psf", bufs=2,
                                    name=f"psf_{pas}_{g}")
                    nc.tensor.matmul(
                        pf[:, :wp], lhsT=fw1_sb[:], rhs=h2fm[:, m0: m0 + wp],
                        start=True, stop=True,
                    )
                    nc.scalar.activation(
                        h3[:, m0: m0 + wp], pf[:, :wp],
                        mybir.ActivationFunctionType.Tanh, bias=fb1_sb[:, :1],
                    )
                    pg = psump.tile([1, 512], mybir.dt.float32, tag="psg", bufs=1,
                                    name=f"psg_{pas}_{g}")
                    nc.tensor.matmul(
                        pg[:, :wp], lhsT=fw2_sb[:], rhs=h3[:, m0: m0 + wp],
                        start=True, stop=True,
                    )
                    ysl = smallp.tile([1, 512], mybir.dt.float32, tag="ysl", bufs=2,
                                      name=f"ysl_{pas}_{g}")
                    nc.scalar.activation(
                        ysl[:, :wp], pg[:, :wp],
                        mybir.ActivationFunctionType.Copy, bias=fb2,
                    )
                    nc.sync.dma_start(out=y_out.ap()[:, m0: m0 + wp], in_=ysl[:, :wp])

                icol = 0
                bcol = 0
                for gi, (sid, blks) in enumerate(groups):
                    nb = len(blks)
                    nidx = nb * P
                    S = nidx // 16
                    stg = gstagep.tile([P, GBLK * 128], mybir.dt.bfloat16, tag="stg",
                                       name=f"stg_{pas}_{gi}")
                    tidx = smallp.tile([P, GBLK * 16], mybir.dt.int16, tag="gidx",
                                       bufs=8, name=f"gidx_{pas}_{gi}")
                    nc.sync.dma_start(out=tidx[:, :S], in_=idx_in.ap()[:, icol: icol + S])
                    nc.gpsimd.dma_gather(
                        stg[:, : nb * 128].rearrange("p (b d) -> p b d", d=128),
                        tabg[sid][:],
                        tidx[:, :S],
                        nidx, nidx, 128,
                        queue_num=gi % GQ,
                    )
                    icol += S
                    # overwrite the even half with the odd half where the
                    # parity mask is set -> stg[:, b, 0:64] holds z2[src]
                    nc.vector.copy_predicated(
                        stg[:, : nb * 128].rearrange("p (b d) -> p b d", d=128)[:, :, 0:64],
                        mask_sb[:, bcol: bcol + nb, None].to_broadcast([P, nb, 64]),
                        stg[:, : nb * 128].rearrange("p (b d) -> p b d", d=128)[:, :, 64:128],
                    )
                    for s0_, e0_ in _segments(blks):
                        k, l = blks[s0_]
                        a0 = l * 64
                        w64 = (e0_ - s0_) * 64
                        nc.vector.tensor_add(
                            acc2[:, a0: a0 + w64].rearrange("p (c f) -> p c f", f=64),
                            acc2[:, a0: a0 + w64].rearrange("p (c f) -> p c f", f=64),
                            stg[:, s0_ * 128: (s0_ + (e0_ - s0_)) * 128].rearrange(
                                "p (b d) -> p b d", d=128
                            )[:, :, 0:64],
                        )
                    bcol += nb
                    for g in st["fc_after"].get(gi, []):
                        emit_fc(g)



    nc.compile()
    return nc


# ---------------------------------------------------------------------------
# Entry point
# ---------------------------------------------------------------------------

def _in_maps(st, per_core, weights):
    xdt = _NPDT[X_DT]
    w1 = np.ascontiguousarray(np.asarray(weights["conv_w1"], np.float32).astype(xdt))
    w2 = np.ascontiguousarray(np.asarray(weights["conv_w2"], np.float32).astype(ml_dtypes.bfloat16))
    fw1 = np.ascontiguousarray(np.asarray(weights["fc_w1"], np.float32).astype(ml_dtypes.bfloat16))
    fw2 = np.ascontiguousarray(np.asarray(weights["fc_w2"], np.float32).astype(ml_dtypes.bfloat16))
    b1 = np.asarray(weights["conv_b1"], np.float32).reshape(64, 1)
    b2e = np.tile(np.asarray(weights["conv_b2"], np.float32)[None, :], (P, 1))
    fb1 = np.asarray(weights["fc_b1"], np.float32).reshape(32, 1)
    maps = []
    for c in range(NCORES):
        pc = per_core[c]
        maps.append(
            {
                "x_exp": pc["x_exp"],
                "idx2": pc["idx"],
                "mask2": np.ascontiguousarray(pc["mask"]),
                "dinv_fm": pc["dinv_fm"],
                "dinv_nm": pc["dinv_nm"],
                "w1": w1,
                "w2": w2,
                "fw1": fw1,
                "fw2": fw2,
                "b1c": b1,
                "b2e": b2e,
                "fb1c": fb1,
            }
        )
    return maps


def kernel(**inputs):
    x = np.asarray(inputs["x"], np.float32)
    edge_index = np.asarray(inputs["edge_index"])
    weights = {
        k: np.asarray(inputs[k], np.float32)
        for k in (
            "conv_w1", "conv_b1", "conv_w2", "conv_b2",
            "fc_w1", "fc_b1", "fc_w2", "fc_b2",
        )
    }
    st, per_core, dinv = _preprocess(x, edge_index)
    nc = _build(st, weights, n_passes=1)
    maps = _in_maps(st, per_core, weights)
    res = None
    for attempt in range(3):
        try:
            res = bass_utils.run_bass_kernel_spmd(
                nc, maps, core_ids=list(range(NCORES))
            )
            break
        except Exception as e:
            if attempt == 2:
                raise
            print(f"[kernel] run attempt {attempt} failed ({e}); retrying")
    N, shard = st["N"], st["shard"]
    node_at = st["node_at"]
    y = np.empty((N, 1), np.float32)
    for c in range(NCORES):
        yc = res.results[c]["y"].reshape(shard)
        valid = node_at[c] >= 0
        y[node_at[c][valid], 0] = yc[valid]
    return y

# revision 38
# speedup vs baseline: 1.0027x; 1.0027x over previous
"""BrainGCN kernel for 8 Trainium2 NeuronCores (Bass/Tile).

Strategy (v3):
- Nodes are degree-sorted and snake-dealt across 8 cores; within a core,
  positions are (degree, stage0-count)-ascending into 49 chunks of 128.
  Chunks are grouped into superchunks of 4 (512 PSUM columns).
- conv1: the host pre-expands x*dinv into per-edge-slot columns (feature
  major, bf16). The device streams slabs and accumulates W1^T @ x_slot
  directly in PSUM across rounds (no DVE reduction at all). tanh/bias/dinv
  applied per superchunk, z2' = (h1*dinv) @ W2 rows written out immediately.
- The bf16 z2' table is AllGathered in two stages (low-degree chunks first,
  so stage 0 ships mid-conv1 and the stage-1 AllGather is hidden under
  stage-0 gathers). Table rows are PAIRS (nodes 2r, 2r+1 -> 256B), so
  dma_gather reads the bf16 table directly (no f32 expansion) and int16
  pair indices reach both stage tables.
- conv2: dma_gather per-edge pair rows, one in-place copy_predicated picks
  the wanted half via a parity mask, then wide strided DVE adds into the
  round-structured accumulator. h2 + the FC head run per 4-chunk group,
  emitted right after the gather group that completes those chunks, so the
  tail is just the last group's chain.

kernel(**inputs) takes FULL inputs, preprocesses + shards on host, compiles
and runs the SPMD program on cores 0..7, and reassembles the full output.
"""

import os
import warnings

warnings.filterwarnings("ignore")

import numpy as np
import ml_dtypes

from concourse import bacc, bass, mybir, tile
from concourse.masks import make_identity
import concourse.bass_utils as bass_utils

P = 128
NCORES = 8
NCH = 49          # chunks per core
# staged table chunk bounds (each stage = contiguous chunk range, superchunk
# aligned, 8*chunks*128 < 32768 rows so int16 gather indices reach them)
SBOUNDS = (0, 20, 49)
NSTG = len(SBOUNDS) - 1
GQ = int(os.environ.get("GCN_GQ", "2"))
GBLK = int(os.environ.get("GCN_GBLK", "8"))   # blocks per dma_gather
SLAB = 4096       # x_exp columns per DMA slab

X_DT = os.environ.get("GCN_XDT", "bf16")      # x_exp dtype: bf16 | fp8
TAB_DT = os.environ.get("GCN_TABDT", "bf16")  # z2 pair-table dtype (bf16: 256B pair rows)

_NPDT = {
    "f32": np.float32,
    "bf16": ml_dtypes.bfloat16,
    "fp8": ml_dtypes.float8_e4m3,
}
_MYDT = {
    "f32": mybir.dt.float32,
    "bf16": mybir.dt.bfloat16,
    "fp8": mybir.dt.float8e4,
}


# ---------------------------------------------------------------------------
# Host preprocessing
# ---------------------------------------------------------------------------

def _preprocess(x, edge_index):
    N = x.shape[0]
    E = edge_index.shape[1]
    src = np.asarray(edge_index[0], dtype=np.int64)
    dst = np.asarray(edge_index[1], dtype=np.int64)

    shard = NCH * P                      # 6272
    percore = N // NCORES                # 6250 (N divisible by 8)
    assert percore * NCORES == N
    assert percore <= shard - NSTG, "need a pad row per stage"
    Hs = [(SBOUNDS[s + 1] - SBOUNDS[s]) * P for s in range(NSTG)]
    assert all(NCORES * h < 32768 for h in Hs)

    deg = 1 + np.bincount(dst, minlength=N)          # includes self-loop
    dinv = (1.0 / np.sqrt(deg)).astype(np.float32)

    # --- snake deal nodes (degree desc) to cores -> equal counts, matched
    # degree profiles
    order = np.argsort(-deg, kind="stable")
    i = np.arange(N)
    pos8 = i % NCORES
    pair = (i // NCORES) % 2
    core_seq = np.where(pair == 0, pos8, NCORES - 1 - pos8)
    core_of = np.empty(N, np.int32)
    core_of[order] = core_seq.astype(np.int32)

    # --- within-core positions: degree ascending; chunk permutation so that
    # within each superchunk of 4 chunks the per-chunk max degree DESCENDS
    # (live chunks at round k form a prefix of the psum tile).
    # prelim chunk q (deg asc) -> device chunk l: reverse within group of 4.
    q_ids = np.arange(NCH)
    g = q_ids // 4
    glen = np.minimum(4, NCH - 4 * g)
    l_of_q = 4 * g + (glen - 1) - (q_ids - 4 * g)

    # slot list in (q asc, lane asc) order, skipping one hole position per
    # stage (device (l=stage_end-1, lane=127) stays empty -> zero table row
    # for padding).
    hole_q = [int(np.where(l_of_q == SBOUNDS[s + 1] - 1)[0][0]) for s in range(NSTG)]
    slot_q = np.repeat(q_ids, P)
    slot_lane = np.tile(np.arange(P), NCH)
    is_hole = np.zeros(len(slot_q), bool)
    for hq in hole_q:
        is_hole |= (slot_q == hq) & (slot_lane == P - 1)
    keep = ~is_hole
    slot_q = slot_q[keep]
    slot_lane = slot_lane[keep]
    slot_l = l_of_q[slot_q]

    l_of = np.empty(N, np.int32)
    lane_of = np.empty(N, np.int32)
    node_at = np.full((NCORES, shard), -1, np.int64)

    def assign_positions(secondary):
        """Place nodes per core ordered by (deg asc, secondary asc)."""
        for c in range(NCORES):
            sel = order[core_of[order] == c]
            key = np.lexsort((secondary[sel], deg[sel]))
            sel = sel[key]
            ls = slot_l[: len(sel)]
            lanes = slot_lane[: len(sel)]
            l_of[sel] = ls
            lane_of[sel] = lanes
            node_at[c, ls * P + lanes] = sel
        return (np.searchsorted(SBOUNDS, l_of, side="right") - 1).astype(np.int8)

    # iteration 1: by degree only -> provisional stages
    stage_of = assign_positions(np.zeros(N, np.int64))
    # iterations 2-3: secondary-sort by stage-0 source count so chunks are
    # homogeneous in per-stage counts (tight conv2 round padding)
    for _ in range(2):
        c0 = np.bincount(dst[stage_of[src] == 0], minlength=N) + (stage_of == 0)
        stage_of = assign_positions(c0)
    pos_of = l_of.astype(np.int64) * P + lane_of

    # --- conv1 block structure: superchunk round-major, live prefix
    # per-chunk max total degree (self included) across all cores
    K1 = np.zeros(NCH, np.int32)
    for c in range(NCORES):
        sel = node_at[c][node_at[c] >= 0]
        np.maximum.at(K1, l_of[sel], deg[sel].astype(np.int32))

    n_sc = (NCH + 3) // 4
    sc_chunks = [list(range(4 * s, min(NCH, 4 * s + 4))) for s in range(n_sc)]
    # per-superchunk rounds: (col0, live) ; colstart2d[k, l] -> x_exp col base
    colstart2d = np.full((int(K1.max()), NCH), -1, np.int64)
    sc_rounds = []   # per sc: list of (col0, ncols)
    cursor = 0
    for s, chs in enumerate(sc_chunks):
        kmax = int(K1[chs[0]])
        assert all(K1[chs[j]] <= K1[chs[j - 1]] for j in range(1, len(chs)))
        rounds = []
        for k in range(kmax):
            live = sum(1 for ch in chs if K1[ch] > k)
            for j in range(live):
                colstart2d[k, chs[j]] = cursor + j * P
            rounds.append((cursor, live * P))
            cursor += live * P
        sc_rounds.append(rounds)
    S1 = cursor

    # slabs: pack rounds into DMA ranges <= SLAB cols; map rounds -> slab;
    # force a slab break at stage boundaries
    stage_end_sc = {SBOUNDS[s + 1] // 4 - 1 for s in range(NSTG)}
    slabs = []      # (col0, col1)
    round_slab = []  # per sc: list of slab idx for each round
    cur0, cur1 = 0, 0
    for s, rounds in enumerate(sc_rounds):
        rs = []
        for (c0, nc_) in rounds:
            if c0 + nc_ - cur0 > SLAB:
                if cur1 > cur0:
                    slabs.append((cur0, cur1))
                cur0 = c0
            cur1 = c0 + nc_
            rs.append(len(slabs))
        round_slab.append(rs)
        if s in stage_end_sc and cur1 > cur0:
            slabs.append((cur0, cur1))
            cur0 = cur1
    if cur1 > cur0:
        slabs.append((cur0, cur1))

    # --- conv2 structure: per-stage round-major blocks
    # counts of stage-s sources per node (self counts in own stage)
    cnt = np.zeros((NSTG, N), np.int64)
    np.add.at(cnt, (stage_of[src], dst), 1)
    for s in range(NSTG):
        cnt[s] += (stage_of == s)
    K2 = np.zeros((NSTG, NCH), np.int32)
    for c in range(NCORES):
        sel = node_at[c][node_at[c] >= 0]
        for s in range(NSTG):
            np.maximum.at(K2[s], l_of[sel], cnt[s][sel].astype(np.int32))

    blocks2 = {s: [] for s in range(NSTG)}
    for s in range(NSTG):
        for k in range(int(K2[s].max())):
            for l in range(NCH):
                if K2[s, l] > k:
                    blocks2[s].append((k, l))
    b2_of = {s: {} for s in range(NSTG)}
    barr = {}
    for s in range(NSTG):
        barr[s] = np.full((max(1, int(K2[s].max())), NCH), -1, np.int64)
        for i_, (k, l) in enumerate(blocks2[s]):
            b2_of[s][(k, l)] = i_
            barr[s][k, l] = i_
    groups = []   # (stage, [block list])
    for s in range(NSTG):
        for i_ in range(0, len(blocks2[s]), GBLK):
            groups.append((s, blocks2[s][i_: i_ + GBLK]))

    slots2 = sum(len(blocks2[s]) for s in range(NSTG)) * P
    per_core_work = (E + N) / NCORES
    print(
        f"[pre] conv1 slots={S1} ({S1/per_core_work:.3f}x) "
        f"conv2 slots={slots2} ({slots2/per_core_work:.3f}x) "
        f"groups={len(groups)} slabs={len(slabs)}"
    )

    # global table rows
    Hs_arr = np.array(Hs)
    base_l = np.array(SBOUNDS[:-1])
    srow = core_of.astype(np.int64) * Hs_arr[stage_of] + (
        (l_of - base_l[stage_of]).astype(np.int64) * P + lane_of
    )

    # --- per-core arrays (vectorized fills)
    xs = x.astype(np.float32) * dinv[:, None]
    xdt = _NPDT[X_DT]
    xsT = np.ascontiguousarray(xs.T).astype(xdt)      # [128, N]

    # edge ranks within destination (self-loop is k=0)
    eorder = np.argsort(dst, kind="stable")
    dst_s = dst[eorder]
    src_s = src[eorder]
    estarts = np.searchsorted(dst_s, np.arange(N))
    k1_e = 1 + np.arange(E) - estarts[dst_s]          # conv1 round of each edge

    # conv2 ranks: within (dst, stage(src)); self first in its own stage
    sstage = stage_of[src_s]
    k2_e = np.empty(E, np.int64)
    for s in range(NSTG):
        m = sstage == s
        d = dst_s[m]
        # rank within equal-d runs (d is sorted)
        startmask = np.ones(len(d), bool)
        startmask[1:] = d[1:] != d[:-1]
        runstart = np.where(startmask, np.arange(len(d)), 0)
        np.maximum.accumulate(runstart, out=runstart)
        k2_e[m] = np.arange(len(d)) - runstart
    # shift by one where the self node is in the same stage as the source
    k2_e += (stage_of[dst_s] == sstage)

    per_core = []
    for c in range(NCORES):
        m = core_of[dst_s] == c
        de, se = dst_s[m], src_s[m]

        # conv1 x_exp
        x_exp = np.zeros((P, S1), dtype=xdt)
        cols_e = colstart2d[k1_e[m], l_of[de]] + lane_of[de]
        own = node_at[c][node_at[c] >= 0]
        cols_self = colstart2d[0, l_of[own]] + lane_of[own]
        x_exp[:, cols_self] = xsT[:, own]
        x_exp[:, cols_e] = xsT[:, se]

        # conv2 idx arrays: gather PAIR rows (256B of bf16 = nodes 2r,2r+1);
        # parity mask selects the wanted half on-device
        idx_parts = []
        par_parts = []
        for s in range(NSTG):
            nb = len(blocks2[s])
            zero_row = c * Hs[s] + Hs[s] - 1
            idxflat = np.full(nb * P, zero_row, np.int64)
            ms = stage_of[se] == s
            bidx = barr[s][k2_e[m][ms], l_of[de[ms]]]
            assert (bidx >= 0).all()
            idxflat[bidx * P + lane_of[de[ms]]] = srow[se[ms]]
            owns = own[stage_of[own] == s]
            bown = barr[s][0, l_of[owns]]
            assert (bown >= 0).all()
            idxflat[bown * P + lane_of[owns]] = srow[owns]
            idx_parts.append(idxflat // 2)
            par_parts.append((idxflat % 2).astype(np.int8))
            assert idx_parts[-1].max() < 32768

        # wrap idx per group: [16, S] slabs concatenated; masks [128, blocks]
        slabs_i = []
        masks_i = []
        for s, blks in groups:
            i0 = b2_of[s][blks[0]]
            flat = idx_parts[s][i0 * P: (i0 + len(blks)) * P]
            S = len(flat) // 16
            slabs_i.append(flat.reshape(S, 16).T.astype(np.int16))
            masks_i.append(par_parts[s][i0 * P: (i0 + len(blks)) * P].reshape(len(blks), P).T)
        idx_cat = np.concatenate(slabs_i, axis=1)
        idx_rep = np.tile(idx_cat, (8, 1))            # [128, sum S]
        mask_cat = np.ascontiguousarray(np.concatenate(masks_i, axis=1))  # [128, nblocks]

        dinv_loc = np.zeros(shard, np.float32)
        vmask = node_at[c] >= 0
        dinv_loc[vmask] = dinv[node_at[c][vmask]]
        dinv_fm = np.tile(dinv_loc[None, :], (64, 1)).astype(ml_dtypes.bfloat16)
        dinv_nm = dinv_loc.reshape(NCH, P).T.astype(np.float32).copy()

        per_core.append(
            dict(x_exp=x_exp, idx=idx_rep, mask=mask_cat, dinv_fm=dinv_fm,
                 dinv_nm=dinv_nm)
        )

    # FC groups of 4 consecutive chunks; emit each group's h2+FC chain right
    # after the gather group that completes its chunks
    last_group_of_chunk = np.zeros(NCH, np.int64)
    for gi, (s, blks) in enumerate(groups):
        for (k, l) in blks:
            last_group_of_chunk[l] = max(last_group_of_chunk[l], gi)
    n_fc = (NCH + 3) // 4
    fc_groups = [list(range(4 * g, min(NCH, 4 * g + 4))) for g in range(n_fc)]
    fc_done = [int(max(last_group_of_chunk[l] for l in fg)) for fg in fc_groups]
    fc_after = {}
    for g, gi in enumerate(fc_done):
        fc_after.setdefault(gi, []).append(g)

    st = dict(
        N=N, shard=shard, Hs=Hs, S1=S1,
        fc_groups=fc_groups, fc_after=fc_after,
        sc_chunks=sc_chunks, sc_rounds=sc_rounds, slabs=slabs,
        round_slab=round_slab, blocks2=blocks2, groups=groups, b2_of=b2_of,
        node_at=node_at, idx_cols=per_core[0]["idx"].shape[1],
        nblocks=per_core[0]["mask"].shape[1],
    )
    return st, per_core, dinv


# ---------------------------------------------------------------------------
# Program builder
# ---------------------------------------------------------------------------

def _segments(blks):
    """Runs of (same k, consecutive l) for wide DVE adds."""
    segs = []
    s = 0
    for i in range(1, len(blks) + 1):
        if (
            i == len(blks)
            or blks[i][0] != blks[s][0]
            or blks[i][1] != blks[i - 1][1] + 1
        ):
            segs.append((s, i))
            s = i
    return segs


def _build(st, weights, n_passes=1):
    shard = st["shard"]
    S1 = st["S1"]
    Hs = st["Hs"]
    groups = st["groups"]
    xdt = _MYDT[X_DT]
    tdt = _MYDT[TAB_DT]
    tnp = _NPDT[TAB_DT]

    fb2 = float(np.asarray(weights["fc_b2"]).reshape(-1)[0])

    nc = bacc.Bacc(
        "TRN2",
        target_bir_lowering=False,
        debug=False,
        enable_asserts=False,
        num_devices=NCORES,
        num_swdge_queues=GQ,
        dynamic_dma_scratch_size=int(os.environ.get("GCN_SCRATCH", "16384")),
    )

    x_exp_in = nc.dram_tensor("x_exp", [P, S1], xdt, kind="ExternalInput")
    idx_in = nc.dram_tensor("idx2", [P, st["idx_cols"]], mybir.dt.int16, kind="ExternalInput")
    mask_in = nc.dram_tensor("mask2", [P, st["nblocks"]], mybir.dt.int8, kind="ExternalInput")
    dinv_fm_in = nc.dram_tensor("dinv_fm", [64, shard], mybir.dt.bfloat16, kind="ExternalInput")
    dinv_nm_in = nc.dram_tensor("dinv_nm", [P, NCH], mybir.dt.float32, kind="ExternalInput")
    w1_in = nc.dram_tensor("w1", [P, 64], xdt, kind="ExternalInput")
    w2_in = nc.dram_tensor("w2", [64, 64], mybir.dt.bfloat16, kind="ExternalInput")
    fw1_in = nc.dram_tensor("fw1", [64, 32], mybir.dt.bfloat16, kind="ExternalInput")
    fw2_in = nc.dram_tensor("fw2", [32, 1], mybir.dt.bfloat16, kind="ExternalInput")
    b1_in = nc.dram_tensor("b1c", [64, 1], mybir.dt.float32, kind="ExternalInput")
    b2e_in = nc.dram_tensor("b2e", [P, 64], mybir.dt.float32, kind="ExternalInput")
    fb1_in = nc.dram_tensor("fb1c", [32, 1], mybir.dt.float32, kind="ExternalInput")
    y_out = nc.dram_tensor("y", [1, shard], mybir.dt.float32, kind="ExternalOutput")

    with tile.TileContext(nc) as tc:
        with (
            tc.tile_pool(name="const", bufs=1) as constp,
            tc.tile_pool(name="big", bufs=1) as bigp,
            tc.tile_pool(name="xslab", bufs=4) as xslabp,
            tc.tile_pool(name="gstage", bufs=7) as gstagep,
            tc.tile_pool(name="psum", bufs=1, space="PSUM") as psump,
            tc.tile_pool(name="small", bufs=2) as smallp,
            tc.tile_pool(name="dram", bufs=1, space="DRAM") as dramp,
        ):
            w1_sb = constp.tile([P, 64], xdt, name="w1_sb")
            nc.sync.dma_start(out=w1_sb[:], in_=w1_in.ap())
            w2_sb = constp.tile([64, 64], mybir.dt.bfloat16, name="w2_sb")
            nc.sync.dma_start(out=w2_sb[:], in_=w2_in.ap())
            fw1_sb = constp.tile([64, 32], mybir.dt.bfloat16, name="fw1_sb")
            nc.sync.dma_start(out=fw1_sb[:], in_=fw1_in.ap())
            fw2_sb = constp.tile([32, 1], mybir.dt.bfloat16, name="fw2_sb")
            nc.sync.dma_start(out=fw2_sb[:], in_=fw2_in.ap())
            b1_sb = constp.tile([64, 1], mybir.dt.float32, name="b1_sb")
            nc.sync.dma_start(out=b1_sb[:], in_=b1_in.ap())
            b2e_sb = constp.tile([P, 64], mybir.dt.float32, name="b2e_sb")
            nc.sync.dma_start(out=b2e_sb[:], in_=b2e_in.ap())
            fb1_sb = constp.tile([32, 1], mybir.dt.float32, name="fb1_sb")
            nc.sync.dma_start(out=fb1_sb[:], in_=fb1_in.ap())
            dinv_fm = constp.tile([64, shard], mybir.dt.bfloat16, name="dinv_fm_sb")
            nc.sync.dma_start(out=dinv_fm[:], in_=dinv_fm_in.ap())
            dinv_nm = constp.tile([P, NCH], mybir.dt.float32, name="dinv_nm_sb")
            nc.sync.dma_start(out=dinv_nm[:], in_=dinv_nm_in.ap())
            ident = constp.tile([P, P], mybir.dt.bfloat16, name="ident")
            make_identity(nc, ident[:])
            mask_sb = constp.tile([P, st["nblocks"]], mybir.dt.int8, name="mask_sb")
            nc.sync.dma_start(out=mask_sb[:], in_=mask_in.ap())


            for pas in range(n_passes):
                # ============ conv1: stream + PSUM accumulate ============
                h1s = bigp.tile([64, shard], mybir.dt.bfloat16, name=f"h1s_{pas}", tag="h1s")
                # pair rows: row r holds nodes (2r, 2r+1), 128 bf16 = 256B
                ag_in = {
                    s: dramp.tile([Hs[s] // 2, 128], tdt, name=f"ag{s}i_{pas}", tag=f"ag{s}i")
                    for s in range(NSTG)
                }
                tabg = {
                    s: dramp.tile([NCORES * Hs[s] // 2, 128], tdt, name=f"t{s}g_{pas}",
                                  tag=f"t{s}g", addr_space="Shared")
                    for s in range(NSTG)
                }

                slabs = st["slabs"]
                slab_tiles = {}

                def get_slab(si):
                    if si not in slab_tiles:
                        c0, c1 = slabs[si]
                        t = xslabp.tile([P, SLAB], xdt, tag="xsl", name=f"xsl_{pas}_{si}")
                        nc.sync.dma_start(out=t[:, : c1 - c0], in_=x_exp_in.ap()[:, c0:c1])
                        slab_tiles[si] = t
                    return slab_tiles[si]

                def emit_z2(s):
                    chs = st["sc_chunks"][s]
                    w = len(chs) * P
                    pz = psump.tile([P, 256], mybir.dt.float32, tag="pz", bufs=1,
                                    name=f"pz_{pas}_{s}")
                    for j, ch in enumerate(chs):
                        nc.tensor.matmul(
                            pz[:, j * 64: (j + 1) * 64],
                            lhsT=h1s[:, ch * P: (ch + 1) * P],
                            rhs=w2_sb[:],
                            start=True, stop=True,
                        )
                    z2sb = smallp.tile([P, 256], tdt, tag="z2sb", name=f"z2sb_{pas}_{s}")
                    nc.scalar.copy(out=z2sb[:, : len(chs) * 64], in_=pz[:, : len(chs) * 64])
                    stg_id = next(i for i in range(NSTG) if chs[0] < SBOUNDS[i + 1])
                    row0 = (chs[0] - SBOUNDS[stg_id]) * P
                    nc.sync.dma_start(
                        out=ag_in[stg_id][row0 // 2: (row0 + w) // 2, :].rearrange(
                            "(c ph) (two f) -> (ph two) c f", ph=64, two=2
                        ),
                        in_=z2sb[:, : len(chs) * 64].rearrange("p (c f) -> p c f", f=64),
                    )

                pending = None
                for s, chs in enumerate(st["sc_chunks"]):
                    w = len(chs) * P
                    pt = psump.tile([64, 512], mybir.dt.float32, tag="acc", bufs=2,
                                    name=f"acc_{pas}_{s}")
                    rounds = st["sc_rounds"][s]
                    nr = len(rounds)
                    for k, (c0, ncol) in enumerate(rounds):
                        si = st["round_slab"][s][k]
                        xsl = get_slab(si)
                        s0 = slabs[si][0]
                        nc.tensor.matmul(
                            pt[:, :ncol],
                            lhsT=w1_sb[:],
                            rhs=xsl[:, c0 - s0: c0 - s0 + ncol],
                            start=(k == 0),
                            stop=(k == nr - 1),
                        )
                    if pending is not None:
                        emit_z2(pending)
                        pending = None
                    base = chs[0] * P
                    tmp = smallp.tile([64, 512], mybir.dt.float32, tag="tmp1",
                                      name=f"tmp1_{pas}_{s}")
                    nc.vector.tensor_mul(tmp[:, :w], pt[:, :w], dinv_fm[:, base: base + w])
                    nc.scalar.activation(
                        tmp[:, :w], tmp[:, :w], mybir.ActivationFunctionType.Tanh,
                        bias=b1_sb[:, :1],
                    )
                    nc.vector.tensor_mul(
                        h1s[:, base: base + w], tmp[:, :w], dinv_fm[:, base: base + w]
                    )
                    if chs[-1] + 1 in SBOUNDS:
                        emit_z2(s)
                        sid = SBOUNDS.index(chs[-1] + 1) - 1
                        nc.gpsimd.collective_compute(
                            "AllGather",
                            mybir.AluOpType.bypass,
                            replica_groups=[list(range(NCORES))],
                            ins=[ag_in[sid].opt()],
                            outs=[tabg[sid].opt()],
                        )
                    else:
                        pending = s

                # scheduler fence: keep conv2 work out of the conv1 streams
                tc.no_sync_barrier()

                # ============ conv2: gather + DVE reduce ============
                acc2 = bigp.tile([P, NCH * 64], mybir.dt.float32, name=f"acc2_{pas}", tag="acc2")
                nc.gpsimd.memset(acc2[:], 0.0)

                h2 = bigp.tile([P, NCH * 64], mybir.dt.bfloat16, name=f"h2_{pas}", tag="h2")
                h2fm = bigp.tile([64, shard], mybir.dt.bfloat16, name=f"h2fm_{pas}", tag="h2fm")
                h3 = bigp.tile([32, shard], mybir.dt.bfloat16, name=f"h3_{pas}", tag="h3")

                def emit_fc(g):
                    chs_f = st["fc_groups"][g]
                    nch_f = len(chs_f)
                    l0 = chs_f[0]
                    a0 = l0 * 64
                    w64 = nch_f * 64
                    m0 = l0 * P
                    wp = nch_f * P
                    # h2 = tanh(acc2*dinv_nm + b2) on this chunk range
                    nc.vector.tensor_mul(
                        acc2[:, a0: a0 + w64].rearrange("p (c f) -> p c f", f=64),
                        acc2[:, a0: a0 + w64].rearrange("p (c f) -> p c f", f=64),
                        dinv_nm[:, l0: l0 + nch_f, None].to_broadcast([P, nch_f, 64]),
                    )
                    nc.vector.tensor_add(
                        acc2[:, a0: a0 + w64].rearrange("p (c f) -> p c f", f=64),
                        acc2[:, a0: a0 + w64].rearrange("p (c f) -> p c f", f=64),
                        b2e_sb[:, None, :].to_broadcast([P, nch_f, 64]),
                    )
                    nc.scalar.activation(
                        h2[:, a0: a0 + w64], acc2[:, a0: a0 + w64],
                        mybir.ActivationFunctionType.Tanh,
                    )
                    for ch in chs_f:
                        ptr = psump.tile([64, P], mybir.dt.bfloat16, tag="pst", bufs=2,
                                         name=f"pst_{pas}_{ch}")
                        nc.tensor.transpose(
                            out=ptr[:], in_=h2[:, ch * 64: (ch + 1) * 64],
                            identity=ident[:],
                        )
                        nc.scalar.copy(out=h2fm[:, ch * P: (ch + 1) * P], in_=ptr[:])
                    pf = psump.tile([32, 512], mybir.dt.float32, tag="psf", bufs=2,
                                    name=f"psf_{pas}_{g}")
                    nc.tensor.matmul(
                        pf[:, :wp], lhsT=fw1_sb[:], rhs=h2fm[:, m0: m0 + wp],
                        start=True, stop=True,
                    )
                    nc.scalar.activation(
                        h3[:, m0: m0 + wp], pf[:, :wp],
                        mybir.ActivationFunctionType.Tanh, bias=fb1_sb[:, :1],
                    )
                    pg = psump.tile([1, 512], mybir.dt.float32, tag="psg", bufs=1,
                                    name=f"psg_{pas}_{g}")
                    nc.tensor.matmul(
                        pg[:, :wp], lhsT=fw2_sb[:], rhs=h3[:, m0: m0 + wp],
                        start=True, stop=True,
                    )
                    ysl = smallp.tile([1, 512], mybir.dt.float32, tag="ysl", bufs=2,
                                      name=f"ysl_{pas}_{g}")
                    nc.scalar.activation(
                        ysl[:, :wp], pg[:, :wp],
                        mybir.ActivationFunctionType.Copy, bias=fb2,
                    )
                    nc.sync.dma_start(out=y_out.ap()[:, m0: m0 + wp], in_=ysl[:, :wp])

                icol = 0
                bcol = 0
                for gi, (sid, blks) in enumerate(groups):
                    nb = len(blks)
                    nidx = nb * P
                    S = nidx // 16
                    stg = gstagep.tile([P, GBLK * 128], mybir.dt.bfloat16, tag="stg",
                                       name=f"stg_{pas}_{gi}")
                    tidx = smallp.tile([P, GBLK * 16], mybir.dt.int16, tag="gidx",
                                       bufs=12, name=f"gidx_{pas}_{gi}")
                    nc.sync.dma_start(out=tidx[:, :S], in_=idx_in.ap()[:, icol: icol + S])
                    nc.gpsimd.dma_gather(
                        stg[:, : nb * 128].rearrange("p (b d) -> p b d", d=128),
                        tabg[sid][:],
                        tidx[:, :S],
                        nidx, nidx, 128,
                        queue_num=gi % GQ,
                    )
                    icol += S
                    # overwrite the even half with the odd half where the
                    # parity mask is set -> stg[:, b, 0:64] holds z2[src]
                    nc.vector.copy_predicated(
                        stg[:, : nb * 128].rearrange("p (b d) -> p b d", d=128)[:, :, 0:64],
                        mask_sb[:, bcol: bcol + nb, None].to_broadcast([P, nb, 64]),
                        stg[:, : nb * 128].rearrange("p (b d) -> p b d", d=128)[:, :, 64:128],
                    )
                    for s0_, e0_ in _segments(blks):
                        k, l = blks[s0_]
                        a0 = l * 64
                        w64 = (e0_ - s0_) * 64
                        nc.vector.tensor_add(
                            acc2[:, a0: a0 + w64].rearrange("p (c f) -> p c f", f=64),
                            acc2[:, a0: a0 + w64].rearrange("p (c f) -> p c f", f=64),
                            stg[:, s0_ * 128: (s0_ + (e0_ - s0_)) * 128].rearrange(
                                "p (b d) -> p b d", d=128
                            )[:, :, 0:64],
                        )
                    bcol += nb
                    for g in st["fc_after"].get(gi, []):
                        emit_fc(g)



    nc.compile()
    return nc


# ---------------------------------------------------------------------------
# Entry point
# ---------------------------------------------------------------------------

def _in_maps(st, per_core, weights):
    xdt = _NPDT[X_DT]
    w1 = np.ascontiguousarray(np.asarray(weights["conv_w1"], np.float32).astype(xdt))
    w2 = np.ascontiguousarray(np.asarray(weights["conv_w2"], np.float32).astype(ml_dtypes.bfloat16))
    fw1 = np.ascontiguousarray(np.asarray(weights["fc_w1"], np.float32).astype(ml_dtypes.bfloat16))
    fw2 = np.ascontiguousarray(np.asarray(weights["fc_w2"], np.float32).astype(ml_dtypes.bfloat16))
    b1 = np.asarray(weights["conv_b1"], np.float32).reshape(64, 1)
    b2e = np.tile(np.asarray(weights["conv_b2"], np.float32)[None, :], (P, 1))
    fb1 = np.asarray(weights["fc_b1"], np.float32).reshape(32, 1)
    maps = []
    for c in range(NCORES):
        pc = per_core[c]
        maps.append(
            {
                "x_exp": pc["x_exp"],
                "idx2": pc["idx"],
                "mask2": np.ascontiguousarray(pc["mask"]),
                "dinv_fm": pc["dinv_fm"],
                "dinv_nm": pc["dinv_nm"],
                "w1": w1,
                "w2": w2,
                "fw1": fw1,
                "fw2": fw2,
                "b1c": b1,
                "b2e": b2e,
                "fb1c": fb1,
            }
        )
    return maps


def kernel(**inputs):
    x = np.asarray(inputs["x"], np.float32)
    edge_index = np.asarray(inputs["edge_index"])
    weights = {
        k: np.asarray(inputs[k], np.float32)
        for k in (
            "conv_w1", "conv_b1", "conv_w2", "conv_b2",
            "fc_w1", "fc_b1", "fc_w2", "fc_b2",
        )
    }
    st, per_core, dinv = _preprocess(x, edge_index)
    nc = _build(st, weights, n_passes=1)
    maps = _in_maps(st, per_core, weights)
    res = None
    for attempt in range(3):
        try:
            res = bass_utils.run_bass_kernel_spmd(
                nc, maps, core_ids=list(range(NCORES))
            )
            break
        except Exception as e:
            if attempt == 2:
                raise
            print(f"[kernel] run attempt {attempt} failed ({e}); retrying")
    N, shard = st["N"], st["shard"]
    node_at = st["node_at"]
    y = np.empty((N, 1), np.float32)
    for c in range(NCORES):
        yc = res.results[c]["y"].reshape(shard)
        valid = node_at[c] >= 0
        y[node_at[c][valid], 0] = yc[valid]
    return y